# revision 37
# baseline (speedup 1.0000x reference)
"""Transformer decoder layer (pre-norm, self-attn + cross-attn + FFN) on 8
Trainium2 NeuronCores.

Sharding: core c handles batch b = c//2 and the contiguous half of the 1024
target tokens h = c%2 (512 query rows each). K/V work is duplicated within
each batch pair so there are no collectives; every core runs an identical
program on different data. The host rotates the token order per core so that
each core's own tokens are always columns [0, 512) -> one uniform SPMD
program.

On-device layout is feature-major ([d_model, token]) throughout. Projections
run in float32r (full PE rate at free dim 512). Attention internals (Q/K
staged, V, scores, exp) run in bf16: full PE rate at ANY free dim, which
enables static causal block-skipping of scores/exp/AV work, and lets both
heads of a feature chunk share one PSUM bank ([128,1024] bf16 = 2KB) so
mask+exp run as single wide ops. The V bias is folded into the output
projection bias on the host, and softmax normalization is applied once per
chunk via a rank-1 broadcast matmul.
"""

import numpy as np
from contextlib import ExitStack

import ml_dtypes
import concourse.bass as bass
import concourse.bacc as bacc
import concourse.tile as tile
from concourse import mybir
from concourse.bass_utils import run_bass_kernel_spmd

# Prefer the act-function table that holds Ln AND Exp (AND Square/Relu/
# Copy/Identity): every activation in this kernel then resolves to ONE
# table and no InstLoadActFuncSet switches appear in the LN chains. The
# emitted act_func_set_id is positional (index into act_info.json), so the
# dict must keep its order; instead, empty every other table so the
# table-load pass can only choose the combined one.
_orig_get_act_tables = bacc.get_activation_tables


def _act_tables_combined_only(arch):
    tabs = _orig_get_act_tables(arch)
    pref = "natural_log_exp_and_others"
    if pref in tabs:
        tabs = {k: (v if k == pref else set()) for k, v in tabs.items()}
    return tabs


bacc.get_activation_tables = _act_tables_combined_only

D = 1024        # d_model
H = 16          # heads
DK = 64         # head dim
DFF = 4096
B = 4
T = 1024        # tgt/src len
OWN = 512       # query rows per core
P = 128         # partitions
NKC = D // P    # 8 feature chunks
NSC = T // P    # 8 s-chunks
NFC = DFF // P  # 32 ffn chunks
EPS = 1e-6

F32 = mybir.dt.float32
F32R = mybir.dt.float32r
BF16 = mybir.dt.bfloat16
AF = mybir.ActivationFunctionType
ALU = mybir.AluOpType


# ---------------------------------------------------------------------------
# program builder (identical for every core; only DRAM contents differ)
# ---------------------------------------------------------------------------

def build_program(repeat=1, mask_mode="causal"):
    nc = bacc.Bacc(None)
    dr = {}

    def din(name, shape, dt=F32):
        dr[name] = nc.dram_tensor(name, list(shape), dt, kind="ExternalInput")
        return dr[name]

    din("xT", [D, T], F32R)                # batch-b x, transposed, own first
    din("memT", [D, T], F32R)              # memory[b] transposed
    if mask_mode == "causal":
        din("diagm", [P, 4 * P])             # [p][i, q] diagonal blocks
    else:
        din("maskT", [NSC, P, OWN])          # full additive mask
    din("sel2", [2, P], F32R)              # rows: [1x64 0x64],[0x64 1x64]
    din("lnwrows", [1, 3 * D], F32R)       # LN w rows, concat (PE bcast)
    # all small per-partition tensors packed into one DMA:
    # cols: [sa_bq 8][sa_bk 8][sa_bo 8][ca_bq 8][ca_bk 8][ca_bo 8]
    #       [b1 32][b2 8][ln1_b 8][ln2_b 8][ln3_b 8][smask 8][tailb 4]
    din("smalls", [P, 124])
    for pre in ("sa", "ca"):
        din(f"{pre}_wq", [NKC, P, D], F32R)
        din(f"{pre}_wk", [NKC, P, D], F32R)
        din(f"{pre}_wv", [NKC, P, D], F32R)  # V weights: W^T row-chunks
        din(f"{pre}_wo", [NKC, P, D], F32R)
    din("w1", [NKC, P, DFF], F32R)
    din("w2", [NFC, P, D], F32R)

    outT = nc.dram_tensor("outT", [D, OWN], F32, kind="ExternalOutput")

    with ExitStack() as ctx:
        tc = ctx.enter_context(tile.TileContext(nc))
        ctx.enter_context(nc.allow_low_precision(
            reason="bf16 attention internals / f32r staging, tol 2e-2"))
        persist = ctx.enter_context(tc.tile_pool(name="persist", bufs=1))

        ones_f = persist.tile([P, 16], F32, tag="ones_f", name="ones_f")
        nc.vector.memset(ones_f[:], 1.0)
        ones_fr = ones_f[:, 0:1].bitcast(F32R)
        sel2 = persist.tile([2, P], F32R, tag="sel2", name="sel2")
        nc.sync.dma_start(sel2[:], dr["sel2"][:])

        smalls = persist.tile([P, 124], F32, tag="smalls", name="smalls")
        nc.gpsimd.dma_start(smalls[:], dr["smalls"][:])
        _off = [0]

        def s_col(n):
            t = smalls[:, _off[0]:_off[0] + n]
            _off[0] += n
            return t

        bias = {}
        for pre in ("sa", "ca"):
            for nm in ("bq", "bk", "bo"):
                bias[f"{pre}_{nm}"] = s_col(NKC)
        bias["b1"] = s_col(NFC)
        bias["b2"] = s_col(NKC)
        lnp = {}
        for ln in ("ln1", "ln2", "ln3"):
            lnp[f"{ln}_b"] = s_col(NKC)
        smask = s_col(NSC)
        tailb = s_col(4)
        lnwr = persist.tile([1, 3 * D], F32R, tag="lnwrows", name="lnwrows")
        nc.gpsimd.dma_start(lnwr[:], dr["lnwrows"][:])
        lnrow = {"ln1": lnwr[0:1, 0:D], "ln2": lnwr[0:1, D:2 * D],
                 "ln3": lnwr[0:1, 2 * D:3 * D]}

        if mask_mode == "causal":
            diagm = persist.tile([P, 4 * P], F32, tag="diagm", name="diagm")
            nc.sync.dma_start(diagm[:], dr["diagm"][:])
            diagm4 = diagm.rearrange("p (i q) -> p i q", i=4)

        # residual (own tokens), lives to the end
        xcur = [persist.tile([P, OWN], F32, tag=f"xc{i}", name=f"xc{i}")
                for i in range(NKC)]

        tmp = ctx.enter_context(tc.tile_pool(name="tmp", bufs=2))
        stats = ctx.enter_context(tc.tile_pool(name="stats", bufs=1))

        # ------------------------------------------------------------------
        def ln_stats(blk, src_get, ps_st):
            """Sum/sumsq matmuls + stats chain. rstd = exp(-0.5 ln(var))
            (= 1/sqrt(var); the reference's eps=1e-6 on std~1 is far below
            the tolerance). Ln/Exp/Square/Relu share one act table, so the
            whole kernel runs without act-table switches. Returns (srcs,
            rstd, mr)."""
            srcs = []
            ps_s = ps_st.tile([1, 512], F32, tag="lns", name="lns")
            ps_q = ps_st.tile([1, 512], F32, tag="lnq", name="lnq")
            for kc in range(NKC):
                sbk = src_get(blk, kc)
                srcs.append(sbk)
                ones_s = ones_fr if sbk.dtype == F32R else ones_f[:, 0:1]
                nc.tensor.matmul(ps_s[:], ones_s, sbk,
                                 start=(kc == 0), stop=(kc == NKC - 1))
                sq = tmp.tile([P, 512], F32R, tag="lnsq", name="lnsq",
                              bufs=1)
                nc.scalar.activation(sq[:], sbk, AF.Square)
                nc.tensor.matmul(ps_q[:], ones_fr, sq[:],
                                 start=(kc == 0), stop=(kc == NKC - 1))
            s2 = stats.tile([1, 512], F32, tag="lnstA", name="lnstA")
            # s2 = (sum/sqrt(D*(D-1)))^2 = sum^2/(D*(D-1))
            nc.scalar.activation(s2[:], ps_s[:], AF.Square,
                                 scale=float(1.0 / np.sqrt(D * (D - 1.0))))
            var = stats.tile([1, 512], F32, tag="lnstB", name="lnstB")
            nc.vector.scalar_tensor_tensor(
                var[:], ps_q[:], 1.0 / (D - 1.0), s2[:],
                op0=ALU.mult, op1=ALU.subtract)
            lnv = stats.tile([1, 512], F32, tag="lnstA", name="lnstA")
            nc.scalar.activation(lnv[:], var[:], AF.Ln)
            rstd = stats.tile([1, 512], F32R, tag="lnstC", name="lnstC")
            nc.scalar.activation(rstd[:], lnv[:], AF.Exp, scale=-0.5)
            mr = stats.tile([1, 512], F32R, tag="lnstB", name="lnstB")
            nc.vector.scalar_tensor_tensor(
                mr[:], ps_s[:], 1.0 / D, rstd[:],
                op0=ALU.mult, op1=ALU.mult)
            return srcs, rstd, mr

        def ln_apply(srcs, rstd, mr, db, wrow, b_pn, ps_st):
            for kc in range(NKC):
                wl = wrow[0:1, kc * P:(kc + 1) * P]
                ps_rb = ps_st.tile([P, 512], F32, tag="ln_rb",
                                   name="ln_rb", bufs=1)
                nc.tensor.matmul(ps_rb[:], wl, rstd[:],
                                 start=True, stop=True)
                ps_mb = ps_st.tile([P, 512], F32, tag="ln_mb",
                                   name="ln_mb", bufs=1)
                nc.tensor.matmul(ps_mb[:], wl, mr[:],
                                 start=True, stop=True)
                t = tmp.tile([P, 512], F32, tag="lnt", name="lnt")
                nc.vector.tensor_mul(t[:], srcs[kc], ps_rb[:])
                nc.vector.scalar_tensor_tensor(
                    db[kc], t[:], b_pn[:, kc:kc + 1], ps_mb[:],
                    op0=ALU.add, op1=ALU.subtract)

        def layer_norm(nblk, src_get, dst_blocks, wrow, b_pn, ps_st,
                       inplace=False):
            """Feature-major LN, h = (x - mean) * (w * rstd) + b. When
            inplace, src_get loads into dst and normalization happens in
            place."""
            del inplace  # srcs returned by ln_stats serve both cases
            for blk in range(nblk):
                srcs, rstd, mr = ln_stats(blk, src_get, ps_st)
                ln_apply(srcs, rstd, mr, dst_blocks[blk], wrow, b_pn, ps_st)

        # ------------------------------------------------------------------
        def load_w_rows(wpool, wname, n=NKC):
            tiles = []
            for kc in range(n):
                wt = wpool.tile([P, D], F32R, tag="w", name="w")
                nc.sync.dma_start(wt[:], dr[wname][kc])
                tiles.append(wt)
            return tiles

        def proj_fm(wpool, wname, bias_pn, src_blocks, dst, ps_acc,
                    out_rows=P):
            """dst[c][:, blk*512:..] = sum_kc W^T[kc,c].T @ src[blk][kc] + b.
            dst tiles may be bf16; DVE casts on the bias add."""
            nblk = len(src_blocks)
            w_tiles = load_w_rows(wpool, wname)
            for c in range(NKC):
                pss = [ps_acc.tile([P, 512], F32, tag=f"proj{blk}",
                                   name=f"proj{blk}") for blk in range(nblk)]
                for kc in range(NKC):
                    for blk in range(nblk):
                        nc.tensor.matmul(pss[blk][:],
                                         w_tiles[kc][:, c * P:(c + 1) * P],
                                         src_blocks[blk][kc],
                                         start=(kc == 0), stop=(kc == NKC - 1))
                for blk in range(nblk):
                    nc.vector.tensor_scalar_add(
                        dst[c][:, blk * 512:(blk + 1) * 512],
                        pss[blk][:], bias_pn[:, c:c + 1])

        def proj_tm_vaug(wpool, wname, src_blocks, vaug, ps_acc):
            """Token-major V projection into [P, H, DK+1] bf16 aug tiles
            (column DK = ones, set by memset outside)."""
            wv = load_w_rows(wpool, wname)
            for dc in range(2):
                for st in range(NSC):
                    sb = src_blocks[st // 4]
                    t0 = (st % 4) * P
                    ps = ps_acc.tile([P, 512], F32, tag="proj0", name="proj0")
                    for kc in range(NKC):
                        nc.tensor.matmul(
                            ps[:], sb[kc][:, t0:t0 + P],
                            wv[kc][:, dc * 512:(dc + 1) * 512],
                            start=(kc == 0), stop=(kc == NKC - 1))
                    nc.vector.tensor_copy(
                        vaug[st][:, 8 * dc:8 * (dc + 1), 0:DK],
                        ps[:].rearrange("p (h d) -> p h d", h=8))

        # ------------------------------------------------------------------
        def attention(KT, QT, vaug, OT, causal, use_smask, att_pools):
            """Scores/exp/AV with bf16 e/V. po rows: [0]=sums (ones column
            first in vaug), [1:65]=unnormalized out. Software-pipelined:
            scores/exp run 2 i-iterations ahead of AV, and chunk c's
            normalize epilogue is emitted mid-way through chunk c+1 so the
            PE never waits on the reciprocal chain."""
            ps_sc, ps_av, epool = att_pools

            def scores_exp(c, i):
                own = causal and i < 4
                q0 = i * P if own else 0
                e = epool.tile([P, 2 * 512], BF16, tag="e", name="e")
                e3 = e.rearrange("p (h q) -> p h q", h=2)
                for h01 in (0, 1):
                    sl = slice(64 * h01, 64 * h01 + 64)
                    sc = ps_sc.tile([P, 512], F32, tag="sc", name="sc")
                    nc.tensor.matmul(
                        sc[:, q0:OWN], KT[c][sl, i * P:(i + 1) * P],
                        QT[c][sl, q0:OWN], start=True, stop=True)
                    if own:
                        # mask only the diagonal 128 strip, in place
                        nc.vector.tensor_tensor(
                            sc[:, q0:q0 + P], sc[:, q0:q0 + P],
                            diagm4[:, i, :], op=ALU.add)
                        nc.scalar.activation(e3[:, h01, q0:OWN],
                                             sc[:, q0:OWN], AF.Exp)
                    elif causal and i >= 4:
                        nc.scalar.activation(e3[:, h01, :], sc[:], AF.Exp,
                                             bias=tailb[:, i - 4:i - 3])
                    elif use_smask:
                        nc.scalar.activation(e3[:, h01, :], sc[:], AF.Exp,
                                             bias=smask[:, i:i + 1])
                    else:
                        nc.vector.tensor_tensor(
                            sc[:], sc[:], mask_tiles[i], op=ALU.add)
                        nc.scalar.activation(e3[:, h01, :], sc[:], AF.Exp)
                return e3, q0

            def av(po, c, i, ei):
                e3, q0 = ei
                for h01 in (0, 1):
                    nc.tensor.matmul(
                        po[h01][:, q0:OWN], vaug[i][:, 2 * c + h01, :],
                        e3[:, h01, q0:OWN],
                        start=(i == 0), stop=(i == NSC - 1),
                        skip_group_check=(q0 != 0))

            def epilogue(prev):
                po, rinv, c = prev
                prb = ps_sc.tile([P, 512], F32, tag="sc", name="sc")
                nc.tensor.matmul(prb[:], sel2[:], rinv[:],
                                 start=True, stop=True)
                rbs = epool.tile([P, 512], F32, tag="rbs", name="rbs",
                                 bufs=2)
                nc.vector.tensor_copy(rbs[:], prb[:])
                nc.vector.tensor_mul(OT[c][0:DK, :], po[0][0:DK, :],
                                     rbs[0:DK, :])
                nc.vector.tensor_mul(OT[c][DK:P, :], po[1][0:DK, :],
                                     rbs[DK:P, :])

            prev = None
            for c in range(NKC):
                po = [ps_av.tile([DK + 1, 512], F32, tag="po0", name="po0"),
                      ps_av.tile([DK + 1, 512], F32, tag="po1", name="po1")]
                eis = {}
                for i in range(NSC):
                    eis[i] = scores_exp(c, i)
                    if i == 3 and prev is not None:
                        epilogue(prev)
                        prev = None
                    if i >= 2:
                        av(po, c, i - 2, eis.pop(i - 2))
                av(po, c, 6, eis.pop(6))
                av(po, c, 7, eis.pop(7))
                # reciprocal can only write partition 0; row 1 of the
                # broadcast rhs is assembled via SBUF->SBUF DMA
                rinv = epool.tile([2, 512], F32R, tag="rinv", name="rinv",
                                  bufs=2)
                r1t = epool.tile([1, 512], F32R, tag="rinv1", name="rinv1",
                                 bufs=2)
                nc.vector.reciprocal(rinv[0:1, :], po[0][DK:DK + 1, :])
                nc.vector.reciprocal(r1t[:], po[1][DK:DK + 1, :])
                nc.sync.dma_start(rinv[1:2, :], r1t[:])
                prev = (po, rinv, c)
            epilogue(prev)

        def out_proj_residual(w_tiles, bias_pn, OT, ps_acc):
            for c2 in range(NKC):
                ps = ps_acc.tile([P, 512], F32, tag="proj0", name="proj0")
                for c in range(NKC):
                    nc.tensor.matmul(ps[:],
                                     w_tiles[c][:, c2 * P:(c2 + 1) * P],
                                     OT[c][:],
                                     start=(c == 0), stop=(c == NKC - 1))
                nc.vector.scalar_tensor_tensor(
                    xcur[c2][:], ps[:], bias_pn[:, c2:c2 + 1], xcur[c2][:],
                    op0=ALU.add, op1=ALU.add)

        def attention_block(prefix, pn, KT, QT, vaug, causal, use_smask,
                            scope):
            """Runs attention + output projection + residual."""
            otp = scope.enter_context(tc.tile_pool(name=f"{pn}_otp", bufs=1))
            OT = [otp.tile([P, OWN], F32R, tag=f"OT{c}", name=f"OT{c}")[:]
                  for c in range(NKC)]
            # wo prefetch overlaps the attention phase
            wsp = scope.enter_context(tc.tile_pool(name=f"{pn}_wso", bufs=8))
            wo_tiles = load_w_rows(wsp, f"{prefix}_wo")
            with ExitStack() as att:
                ps_sc = att.enter_context(
                    tc.tile_pool(name=f"{pn}_psc", bufs=4, space="PSUM"))
                ps_av = att.enter_context(
                    tc.tile_pool(name=f"{pn}_pav", bufs=2, space="PSUM"))
                epool = att.enter_context(
                    tc.tile_pool(name=f"{pn}_ep", bufs=4))
                attention(KT, QT, vaug, OT, causal, use_smask,
                          (ps_sc, ps_av, epool))
            with tc.tile_pool(name=f"{pn}_pso", bufs=4, space="PSUM") as pso:
                out_proj_residual(wo_tiles, bias[f"{prefix}_bo"], OT, pso)

        mask_tiles = None
        for _rep in range(repeat):
            # ==============================================================
            # phase 1+2: LN1, self-attention
            # ==============================================================
            with ExitStack() as sa:
                big = sa.enter_context(
                    tc.tile_pool(name=f"sa_big{_rep}", bufs=1))
                QT = [big.tile([P, OWN], BF16, tag=f"QT{c}", name=f"QT{c}")[:]
                      for c in range(NKC)]
                KT = [big.tile([P, T], BF16, tag=f"KT{c}", name=f"KT{c}")[:]
                      for c in range(NKC)]
                vaug = [big.tile([P, H, DK + 1], BF16, tag=f"V{i}",
                                 name=f"V{i}")[:] for i in range(NSC)]
                for i in range(NSC):
                    nc.vector.tensor_copy(vaug[i][:, :, DK], ones_f[:, :])

                with ExitStack() as ph:
                    h1p = ph.enter_context(
                        tc.tile_pool(name=f"h1{_rep}", bufs=1))
                    h1_blocks = [
                        [h1p.tile([P, 512], F32R, tag=f"h1_{blk}_{kc}",
                                  name=f"h1_{blk}_{kc}")[:]
                         for kc in range(NKC)]
                        for blk in range(2)]
                    with ExitStack() as wscope:
                        # weight pool open during LN1 so the Q/K/V weight
                        # prefetch overlaps the LN chain
                        wsp = wscope.enter_context(
                            tc.tile_pool(name=f"sa_ws{_rep}", bufs=9))
                        psa = wscope.enter_context(
                            tc.tile_pool(name=f"ps_sap{_rep}", bufs=2,
                                         space="PSUM"))
                        with ExitStack() as lnscope:
                            ps_ln = lnscope.enter_context(
                                tc.tile_pool(name=f"ps_ln1{_rep}", bufs=1,
                                             space="PSUM"))

                            def x_get(blk, kc):
                                dst = h1_blocks[blk][kc]
                                nc.sync.dma_start(
                                    dst,
                                    dr["xT"][kc * P:(kc + 1) * P,
                                             blk * 512:(blk + 1) * 512])
                                return dst

                            s0 = ln_stats(0, x_get, ps_ln)
                            ln_apply(*s0, h1_blocks[0], lnrow["ln1"],
                                     lnp["ln1_b"], ps_ln)
                            s1 = ln_stats(1, x_get, ps_ln)
                            # Q proj only needs block 0: it overlaps the
                            # block-1 normalize chain
                            proj_fm(wsp, "sa_wq", bias["sa_bq"],
                                    [h1_blocks[0]], QT, psa)
                            ln_apply(*s1, h1_blocks[1], lnrow["ln1"],
                                     lnp["ln1_b"], ps_ln)
                        proj_fm(wsp, "sa_wk", bias["sa_bk"], h1_blocks,
                                KT, psa)
                        proj_tm_vaug(wsp, "sa_wv", h1_blocks, vaug, psa)

                # residual base (own half of x)
                for kc in range(NKC):
                    nc.sync.dma_start(
                        xcur[kc][:],
                        dr["xT"][kc * P:(kc + 1) * P, 0:OWN].bitcast(F32))

                with ExitStack() as mscope:
                    if mask_mode != "causal":
                        mp = mscope.enter_context(
                            tc.tile_pool(name=f"maskp{_rep}", bufs=1))
                        mask_tiles = []
                        for i in range(NSC):
                            mt = mp.tile([P, OWN], F32, tag=f"mask{i}",
                                         name=f"mask{i}")
                            nc.sync.dma_start(mt[:], dr["maskT"][i])
                            mask_tiles.append(mt[:])
                    attention_block("sa", f"sa{_rep}", KT, QT, vaug,
                                    mask_mode == "causal", False, mscope)

            # ==============================================================
            # phase 3: cross attention
            # ==============================================================
            with ExitStack() as ca:
                big = ca.enter_context(
                    tc.tile_pool(name=f"ca_big{_rep}", bufs=1))
                QT = [big.tile([P, OWN], BF16, tag=f"cQT{c}",
                               name=f"cQT{c}")[:] for c in range(NKC)]
                KT = [big.tile([P, T], BF16, tag=f"cKT{c}",
                               name=f"cKT{c}")[:] for c in range(NKC)]
                vaug = [big.tile([P, H, DK + 1], BF16, tag=f"cV{i}",
                                 name=f"cV{i}")[:] for i in range(NSC)]
                for i in range(NSC):
                    nc.vector.tensor_copy(vaug[i][:, :, DK], ones_f[:, :])

                # K/V projections first: they depend only on memory, so
                # they keep the PE busy while the LN2 chain runs.
                with ExitStack() as mm_scope:
                    wsp = mm_scope.enter_context(
                        tc.tile_pool(name=f"ca_wkv{_rep}", bufs=9))
                    pca = mm_scope.enter_context(
                        tc.tile_pool(name=f"ps_ckv{_rep}", bufs=2,
                                     space="PSUM"))
                    with ExitStack() as ph:
                        h2p = ph.enter_context(
                            tc.tile_pool(name=f"h2{_rep}", bufs=1))
                        h2 = [h2p.tile([P, OWN], F32R, tag=f"h2_{kc}",
                                       name=f"h2_{kc}")[:]
                              for kc in range(NKC)]
                        psl = ph.enter_context(
                            tc.tile_pool(name=f"ps_ln2{_rep}", bufs=1,
                                         space="PSUM"))
                        # LN2 stats first: the K/V projections (which only
                        # need memory) keep the PE busy through the serial
                        # stats chain; ln_apply lands after them.
                        ln2_stats = ln_stats(0, lambda blk, kc: xcur[kc][:],
                                             psl)
                        with ExitStack() as mscope2:
                            memp = mscope2.enter_context(
                                tc.tile_pool(name=f"mem{_rep}", bufs=1))
                            mem_blocks = []
                            for blk in range(2):
                                mb = []
                                for kc in range(NKC):
                                    mt = memp.tile([P, 512], F32R,
                                                   tag=f"m{blk}_{kc}",
                                                   name=f"m{blk}_{kc}")
                                    nc.sync.dma_start(
                                        mt[:],
                                        dr["memT"][kc * P:(kc + 1) * P,
                                                   blk * 512:
                                                   (blk + 1) * 512])
                                    mb.append(mt[:])
                                mem_blocks.append(mb)
                            proj_fm(wsp, "ca_wk", bias["ca_bk"], mem_blocks,
                                    KT, pca)
                            proj_tm_vaug(wsp, "ca_wv", mem_blocks, vaug,
                                         pca)
                        ln_apply(*ln2_stats, h2, lnrow["ln2"],
                                 lnp["ln2_b"], psl)
                        proj_fm(wsp, "ca_wq", bias["ca_bq"], [h2], QT, pca)

                with ExitStack() as ascope:
                    attention_block("ca", f"ca{_rep}", KT, QT, vaug, False,
                                    True, ascope)

            # ==============================================================
            # phase 4: FFN
            # ==============================================================
            with ExitStack() as ff:
                ap_pool = ff.enter_context(
                    tc.tile_pool(name=f"aT{_rep}", bufs=1))
                w2p = ff.enter_context(tc.tile_pool(name=f"w2p{_rep}",
                                                    bufs=8))
                aT = [ap_pool.tile([P, OWN], F32R, tag=f"aT{i}",
                                   name=f"aT{i}")[:] for i in range(NFC)]
                with ExitStack() as ph:
                    h3p = ph.enter_context(
                        tc.tile_pool(name=f"h3{_rep}", bufs=1))
                    h3 = [h3p.tile([P, OWN], F32R, tag=f"h3_{kc}",
                                   name=f"h3_{kc}")[:] for kc in range(NKC)]
                    with ExitStack() as wscope:
                        wsp = wscope.enter_context(
                            tc.tile_pool(name=f"ff_ws{_rep}", bufs=12))
                        ps_f1 = wscope.enter_context(
                            tc.tile_pool(name=f"ps_ff1{_rep}", bufs=4,
                                         space="PSUM"))
                        with tc.tile_pool(name=f"ps_ln3{_rep}", bufs=1,
                                          space="PSUM") as psl:
                            layer_norm(1, lambda blk, kc: xcur[kc][:], [h3],
                                       lnrow["ln3"], lnp["ln3_b"], psl)
                        for cg in range(4):
                            w1t = []
                            for kc in range(NKC):
                                wt = wsp.tile([P, 1024], F32R, tag="w",
                                              name="w")
                                nc.sync.dma_start(
                                    wt[:], dr["w1"][kc][:, cg * 1024:
                                                        (cg + 1) * 1024])
                                w1t.append(wt)
                            for cc in range(NKC):
                                cidx = cg * 8 + cc
                                ps = ps_f1.tile([P, 512], F32, tag="proj0",
                                                name="proj0")
                                for kc in range(NKC):
                                    nc.tensor.matmul(
                                        ps[:], w1t[kc][:, cc * P:(cc + 1) * P],
                                        h3[kc], start=(kc == 0),
                                        stop=(kc == NKC - 1))
                                nc.scalar.activation(
                                    aT[cidx], ps[:], AF.Relu,
                                    bias=bias["b1"][:, cidx:cidx + 1])

                with ExitStack() as yscope:
                    ps_y = yscope.enter_context(
                        tc.tile_pool(name=f"ps_y{_rep}", bufs=1,
                                     space="PSUM"))
                    yps = [ps_y.tile([P, 512], F32, tag=f"y{c2}",
                                     name=f"y{c2}") for c2 in range(NKC)]
                    for kc2 in range(NFC):
                        wt = w2p.tile([P, D], F32R, tag="w", name="w")
                        nc.sync.dma_start(wt[:], dr["w2"][kc2])
                        for c2 in range(NKC):
                            nc.tensor.matmul(
                                yps[c2][:], wt[:, c2 * P:(c2 + 1) * P],
                                aT[kc2], start=(kc2 == 0),
                                stop=(kc2 == NFC - 1))
                    for c2 in range(NKC):
                        nc.vector.scalar_tensor_tensor(
                            xcur[c2][:], yps[c2][:], bias["b2"][:, c2:c2 + 1],
                            xcur[c2][:], op0=ALU.add, op1=ALU.add)

            for c2 in range(NKC):
                nc.sync.dma_start(outT[c2 * P:(c2 + 1) * P, :], xcur[c2][:])

    nc.finalize()
    return nc


# ---------------------------------------------------------------------------
# host side
# ---------------------------------------------------------------------------

def host_prep(inputs):
    f32 = np.float32
    bf16 = ml_dtypes.bfloat16
    x = np.asarray(inputs["x"], f32)
    mem = np.asarray(inputs["memory"], f32)
    tgt = np.asarray(inputs["tgt_mask"])
    src = np.asarray(inputs["src_mask"])

    add_tgt = (tgt.astype(f32) - 1.0) * 1e9     # [B, T, T]: 0 or -1e9
    add_src = (src.astype(f32) - 1.0) * 1e9     # [B, T]

    shared = {}
    sel2 = np.zeros((2, P), f32)
    sel2[0, 0:DK] = 1.0
    sel2[1, DK:P] = 1.0
    shared["sel2"] = sel2
    sm_cols = []
    for pre in ("sa", "ca"):
        wq = np.asarray(inputs[f"{pre}_wq"], f32)
        wo = np.asarray(inputs[f"{pre}_wo"], f32)
        bv = np.asarray(inputs[f"{pre}_bv"], f32)
        bo = np.asarray(inputs[f"{pre}_bo"], f32)
        shared[f"{pre}_wq"] = np.ascontiguousarray(
            (wq.T * 0.125).reshape(NKC, P, D))
        shared[f"{pre}_wk"] = np.ascontiguousarray(
            np.asarray(inputs[f"{pre}_wk"], f32).T.reshape(NKC, P, D))
        shared[f"{pre}_wo"] = np.ascontiguousarray(wo.T.reshape(NKC, P, D))
        shared[f"{pre}_wv"] = np.ascontiguousarray(
            np.asarray(inputs[f"{pre}_wv"], f32).T.reshape(NKC, P, D))
        # fold the V bias through attention into the output projection bias
        bo_folded = bo + wo @ bv
        sm_cols.append((np.asarray(inputs[f"{pre}_bq"], f32)
                        * 0.125).reshape(NKC, P).T)
        sm_cols.append(np.asarray(inputs[f"{pre}_bk"], f32).reshape(NKC, P).T)
        sm_cols.append(bo_folded.reshape(NKC, P).T)
    shared["lnwrows"] = np.concatenate(
        [np.asarray(inputs[f"{ln}_w"], f32) for ln in
         ("ln1", "ln2", "ln3")]).reshape(1, 3 * D)
    shared["w1"] = np.ascontiguousarray(
        np.asarray(inputs["ff_w1"], f32).T.reshape(NKC, P, DFF))
    shared["w2"] = np.ascontiguousarray(
        np.asarray(inputs["ff_w2"], f32).T.reshape(NFC, P, D))
    sm_fixed = [sm_cols[0], sm_cols[1], sm_cols[2],
                sm_cols[3], sm_cols[4], sm_cols[5],
                np.asarray(inputs["ff_b1"], f32).reshape(NFC, P).T,
                np.asarray(inputs["ff_b2"], f32).reshape(NKC, P).T]
    for ln in ("ln1", "ln2", "ln3"):
        sm_fixed.append(np.asarray(inputs[f"{ln}_b"], f32).reshape(NKC, P).T)

    maps = []
    for c in range(8):
        b, half = c // 2, c % 2
        q0 = half * OWN
        order = np.concatenate(
            [np.arange(q0, q0 + OWN), np.r_[0:q0, q0 + OWN:T]]).astype(
                np.int64)
        m = dict(shared)
        m["xT"] = np.ascontiguousarray(x[b][order].T)
        m["memT"] = np.ascontiguousarray(mem[b].T)
        mt = np.ascontiguousarray(add_tgt[b][q0:q0 + OWN][:, order].T)
        m["_maskT_rot"] = mt           # [s_rot, q_local], stripped later
        smask_c = add_src[b].reshape(NSC, P).T
        tailb_c = mt[OWN:, 0].reshape(4, P).T
        m["smalls"] = np.ascontiguousarray(
            np.concatenate(sm_fixed + [smask_c, tailb_c], axis=1))
        maps.append(m)
    return maps


def classify_mask(maps):
    """'causal' when every core's rotated mask has: constant tail rows
    (chunks 4..7), fully-masked blocks strictly below the 128-diagonal, and
    fully-open blocks strictly above it. Else 'full'."""
    for m in maps:
        mt = m["_maskT_rot"]           # [T, OWN] additive
        tail = mt[OWN:, :]
        if not np.all(tail == tail[:, :1]):
            return "full"
        own = mt[:OWN, :]              # [s, q]
        for i in range(4):
            blk = own[i * P:(i + 1) * P]
            if not np.all(blk[:, :i * P] == -1e9):
                return "full"
            if not np.all(blk[:, (i + 1) * P:] == 0.0):
                return "full"
            dg = blk[:, i * P:(i + 1) * P]
            expect = np.where(np.arange(P)[:, None] <= np.arange(P)[None, :],
                              0.0, -1e9).astype(np.float32)
            if not np.array_equal(dg, expect):
                return "full"
    return "causal"


def finalize_maps(maps, mode):
    for m in maps:
        mt = m.pop("_maskT_rot")
        if mode == "causal":
            dg = np.zeros((P, 4, P), np.float32)
            for i in range(4):
                dg[:, i, :] = mt[i * P:(i + 1) * P, i * P:(i + 1) * P]
            m["diagm"] = np.ascontiguousarray(dg.reshape(P, 4 * P))
        else:
            m["maskT"] = np.ascontiguousarray(mt.reshape(NSC, P, OWN))
    return maps


def gather(results):
    out = np.zeros((B, T, D), np.float32)
    for c in range(8):
        b, half = c // 2, c % 2
        out[b, half * OWN:(half + 1) * OWN, :] = results[c]["outT"].T
    return out


_NC_CACHE = {}


def kernel(**inputs):
    in_maps = host_prep(inputs)
    mode = classify_mask(in_maps)
    in_maps = finalize_maps(in_maps, mode)
    if mode not in _NC_CACHE:
        _NC_CACHE[mode] = build_program(mask_mode=mode)
    nc = _NC_CACHE[mode]
    res = run_bass_kernel_spmd(nc, in_maps, list(range(8)))
    return gather(res.results)


if __name__ == "__main__":
    import reference as ref_mod
    inputs = {k: np.asarray(v) for k, v in ref_mod.setup_inputs().items()}
    expected = np.asarray(ref_mod.reference(**ref_mod.setup_inputs()))
    actual = kernel(**inputs)
    err = np.abs(actual - expected).max()
    rel = err / np.abs(expected).max()
    print("max abs err:", err, "rel:", rel)
